# revision 1
# baseline (speedup 1.0000x reference)
"""CGConvBlock (3x CGConv + MLP/BatchNorm + graph LayerNorm) on 8 Trainium2 cores.

Sharding: nodes by graph (4 graphs/core, per-graph padded to GPAD rows);
edges by destination core (sorted by padded dst). Per layer:
  - gather x[dst], x[src] (bf16, feature-major) via transposed dma_gather
  - gate/core pre-acts: z-chunk-stationary bf16 matmuls -> PSUM [edge, 256]
  - msg = softplus(core) * sigmoid(gate) via Exp/Ln table + DVE reciprocal
  - scatter-add: matmul(lhsT=msg[e,c], rhs=onehot[e,n]) accumulated per
    128-node block in PSUM -> [c, n] aggregation
  - MLP with global BatchNorm (stats allreduced), residuals in fp32
  - per-graph LayerNorm with static segments (graph-padded layout)
  - AllGather of the bf16 node table for the next layer's gathers
"""
import sys

sys.path.insert(0, "/opt/trn_rl_repo")

import numpy as np
import ml_dtypes

N = 20000
E = 640000
C = 128
D = 64
H = 512
L = 3
G = 32
EPS = 1e-5
NCORES = 8
GPG = G // NCORES  # graphs per core = 4
SG = 6             # edge tiles per activation supergroup
NCH = 512          # node free-dim chunk for MLP/LN

BF16 = ml_dtypes.bfloat16


def _wrap_idx(idx):
    """[n] int -> [16, n//16] int16 in the gather engine's wrapped layout
    (replicated to 128 partitions on device)."""
    n = idx.shape[0]
    assert n % 16 == 0
    return np.ascontiguousarray(idx.reshape(n // 16, 16).T.astype(np.int16))


def _preprocess(x, node_batch, edge_index, edge_attr):
    nb = np.asarray(node_batch)
    ei = np.asarray(edge_index)
    NN = nb.shape[0]
    counts = np.bincount(nb, minlength=G)
    gstart = np.concatenate([[0], np.cumsum(counts)[:-1]])
    GPAD = max(128, int(np.ceil(counts.max() / 128)) * 128)
    NPAD = GPG * GPAD
    NB = NPAD // 128
    PTOT = NCORES * NPAD
    assert PTOT < 32768

    g_of = nb  # nodes sorted by graph
    core_of_node = g_of // GPG
    slot = (g_of % GPG) * GPAD + (np.arange(NN) - gstart[g_of])
    pad_slot = core_of_node * NPAD + slot  # global padded index

    src, dst = ei[0], ei[1]
    core_of_edge = core_of_node[dst]
    dst_local = pad_slot[dst] - core_of_edge * NPAD

    # per (core, block) edge lists
    order = np.lexsort((dst_local, core_of_edge))
    src_s, dst_s = src[order], dst_local[order]
    core_s = core_of_edge[order]
    blk_s = dst_s // 128
    # counts[core, block]
    cb = np.zeros((NCORES, NB), np.int64)
    np.add.at(cb, (core_s, blk_s), 1)
    T_b = [int(np.ceil(cb[:, b].max() / 128)) for b in range(NB)]
    TT = sum(T_b)
    EP = TT * 128

    core_edge_start = np.concatenate([[0], np.cumsum(np.bincount(core_s, minlength=NCORES))])
    per_core = []
    x_np = np.asarray(x)
    ea = np.asarray(edge_attr)

    for c in range(NCORES):
        lo, hi = core_edge_start[c], core_edge_start[c + 1]
        srcc, dstc, blkc = src_s[lo:hi], dst_s[lo:hi], blk_s[lo:hi]
        eidc = order[lo:hi]
        # slot edges into padded per-block tile space
        src_pad = np.zeros(EP, np.int64)
        dstrel = np.full(EP, -1.0, np.float32)
        attr_sel = np.zeros(EP, np.int64)
        attr_valid = np.zeros(EP, bool)
        off = 0
        boff = np.concatenate([[0], np.cumsum(np.bincount(blkc, minlength=NB))])
        for b in range(NB):
            cnt = boff[b + 1] - boff[b]
            sl = slice(boff[b], boff[b + 1])
            src_pad[off:off + cnt] = pad_slot[srcc[sl]]
            dstrel[off:off + cnt] = (dstc[sl] - 128 * b).astype(np.float32)
            attr_sel[off:off + cnt] = eidc[sl]
            attr_valid[off:off + cnt] = True
            off += T_b[b] * 128
        assert off == EP
        dst_pad = np.zeros(EP, np.int64)
        m = dstrel >= 0
        dst_pad[m] = (dstrel[m] + 128 * np.repeat(np.arange(NB), np.array(T_b) * 128)[m]).astype(np.int64)

        attrT = np.zeros((65, EP), BF16)
        attrT[:D, attr_valid] = ea[attr_sel[attr_valid]].astype(BF16).T
        attrT[D:, :] = 0
        attrT[64, :] = BF16(1.0)

        xcn = np.zeros((C, NPAD), np.float32)
        mask = np.zeros((1, NPAD), np.float32)
        own = core_of_node == c
        xcn[:, slot[own]] = x_np[own].T
        mask[0, slot[own]] = 1.0
        xncbf = np.zeros((NPAD, C), BF16)
        xncbf[slot[own]] = x_np[own].astype(BF16)
        invcnt = (1.0 / (np.maximum(counts[c * GPG:(c + 1) * GPG], 1) * C)).astype(np.float32).reshape(1, GPG)

        per_core.append(dict(
            srcidx=_wrap_idx(src_pad),
            dstidx=_wrap_idx(dst_pad),
            dstrel=np.ascontiguousarray(dstrel.reshape(TT, 128).T),  # [128, TT]
            attrt=attrT,
            xcn=xcn,
            mask=np.ascontiguousarray(np.broadcast_to(mask, (128, NPAD))),
            invcnt=invcnt,
            xncbf=xncbf,
        ))

    meta = dict(GPAD=GPAD, NPAD=NPAD, NB=NB, PTOT=PTOT, T_b=T_b, TT=TT, EP=EP,
                pad_slot=pad_slot, counts=counts, NN=NN)
    return per_core, meta


def _prep_weights(Wf, bf, Ws, bs, W1, b1, g1, be1, W2, b2, lnw, lnb):
    wz = np.zeros((L, 3, 128, 2 * C), np.float32)
    for l in range(L):
        wcat = np.concatenate([-Wf[l].T, Ws[l].T], axis=1)  # [Z, 2C] (gate negated)
        wz[l, 0, :, :] = wcat[0:128]
        wz[l, 1, :, :] = wcat[128:256]
        wz[l, 2, :D, :] = wcat[256:320]
        wz[l, 2, 64, :] = np.concatenate([-bf[l], bs[l]])
    w1t = np.stack([np.stack([W1[l].T[:, 128 * k:128 * (k + 1)] for k in range(4)]) for l in range(L)])
    w2t = np.stack([np.stack([W2[l].T[128 * k:128 * (k + 1), :] for k in range(4)]) for l in range(L)])
    return dict(
        wz=wz.reshape(L * 3, 128, 2 * C).astype(BF16),
        w1t=w1t.reshape(L * 4, 128, 128).astype(BF16),
        w2t=w2t.reshape(L * 4, 128, 128).astype(BF16),
        b1c=np.ascontiguousarray(np.asarray(b1, np.float32).reshape(L, 4, 128).transpose(2, 0, 1).reshape(128, L * 4)),
        g1c=np.ascontiguousarray(np.asarray(g1, np.float32).reshape(L, 4, 128).transpose(2, 0, 1).reshape(128, L * 4)),
        be1c=np.ascontiguousarray(np.asarray(be1, np.float32).reshape(L, 4, 128).transpose(2, 0, 1).reshape(128, L * 4)),
        b2c=np.ascontiguousarray(np.asarray(b2, np.float32).T),     # [128, L]
        lnwr=np.asarray(lnw, np.float32).reshape(1, L * 128),
        lnbr=np.asarray(lnb, np.float32).reshape(1, L * 128),
    )


def _trace(meta, nlayers=L, use_cc=True, edge_only=False):
    from concourse import bacc, mybir
    import concourse.tile as tile

    # Force every activation onto the exp+ln table (index 6) so the ACT
    # table-load pass never alternates tables between Exp and Ln ops.
    import concourse.hw_specs as _hw
    if not hasattr(bacc, "_orig_get_act_tables"):
        bacc._orig_get_act_tables = bacc.get_activation_tables

        def _only_table6(arch):
            tabs = bacc._orig_get_act_tables(arch)
            out = {}
            for i, (name, funcs) in enumerate(tabs.items()):
                out[name] = funcs if name == "natural_log_exp_and_others" else set()
            return out

        bacc.get_activation_tables = _only_table6

    F32 = mybir.dt.float32
    B16 = mybir.dt.bfloat16
    I16 = mybir.dt.int16
    AF = mybir.ActivationFunctionType
    OP = mybir.AluOpType

    NPAD, NB, PTOT, TT, EP = meta["NPAD"], meta["NB"], meta["PTOT"], meta["TT"], meta["EP"]
    GPAD = meta["GPAD"]
    T_b = meta["T_b"]
    NNCH = (NPAD + NCH - 1) // NCH  # node chunks (NPAD % 512 may be 256)

    nc = bacc.Bacc("TRN2", target_bir_lowering=False, debug=False, num_devices=NCORES)

    def din(name, shape, dt):
        return nc.dram_tensor(name, shape, dt, kind="ExternalInput").ap()

    xcn_in = din("xcn", [C, NPAD], F32)
    xncbf_in = din("xncbf", [NPAD, C], B16)
    srcidx_in = din("srcidx", [16, TT * 8], I16)
    dstidx_in = din("dstidx", [16, TT * 8], I16)
    dstrel_in = din("dstrel", [128, TT], F32)
    attrt_in = din("attrt", [65, EP], B16)
    mask_in = din("mask", [128, NPAD], F32)
    invcnt_in = din("invcnt", [1, GPG], F32)
    iota_in = din("iota", [128, SG * 128], B16)
    ident_in = din("ident", [128, 128], F32)
    ones_in = din("ones", [1, NPAD], F32)
    wz_in = din("wz", [L * 3, 128, 2 * C], B16)
    w1t_in = din("w1t", [L * 4, 128, 128], B16)
    w2t_in = din("w2t", [L * 4, 128, 128], B16)
    b1c_in = din("b1c", [128, L * 4], F32)
    g1c_in = din("g1c", [128, L * 4], F32)
    be1c_in = din("be1c", [128, L * 4], F32)
    b2c_in = din("b2c", [128, L], F32)
    lnwr_in = din("lnwr", [1, L * 128], F32)
    lnbr_in = din("lnbr", [1, L * 128], F32)
    xout = nc.dram_tensor("xout", [NPAD, C], F32, kind="ExternalOutput").ap()

    with tile.TileContext(nc) as tc:
        with (
            tc.tile_pool(name="const", bufs=1) as cp,
            tc.tile_pool(name="xstate", bufs=2) as xp,
            tc.tile_pool(name="dram", bufs=1, space="DRAM") as dr,
        ):
            # ---- constants ----
            srcidx = cp.tile([128, TT * 8], I16)
            dstidx = cp.tile([128, TT * 8], I16)
            for r in range(8):
                nc.sync.dma_start(out=srcidx[16 * r:16 * (r + 1), :], in_=srcidx_in[:])
                nc.sync.dma_start(out=dstidx[16 * r:16 * (r + 1), :], in_=dstidx_in[:])
            dstrel = cp.tile([128, TT], F32)
            nc.sync.dma_start(out=dstrel[:], in_=dstrel_in[:])
            mask = cp.tile([128, NPAD], F32)
            nc.sync.dma_start(out=mask[:], in_=mask_in[:])
            invcnt = cp.tile([1, GPG], F32)
            nc.sync.dma_start(out=invcnt[:], in_=invcnt_in[:])
            iota = cp.tile([128, SG * 128], B16)
            nc.sync.dma_start(out=iota[:], in_=iota_in[:])
            ident = cp.tile([128, 128], F32)
            nc.sync.dma_start(out=ident[:], in_=ident_in[:])
            ones = cp.tile([1, NPAD], F32)
            nc.sync.dma_start(out=ones[:], in_=ones_in[:])
            wz = cp.tile([128, L * 3, 2 * C], B16)
            for i in range(L * 3):
                nc.sync.dma_start(out=wz[:, i, :], in_=wz_in[i])
            w1t = cp.tile([128, L * 4, 128], B16)
            w2t = cp.tile([128, L * 4, 128], B16)
            for i in range(L * 4):
                nc.sync.dma_start(out=w1t[:, i, :], in_=w1t_in[i])
                nc.sync.dma_start(out=w2t[:, i, :], in_=w2t_in[i])
            b1c = cp.tile([128, L * 4], F32)
            nc.sync.dma_start(out=b1c[:], in_=b1c_in[:])
            g1c = cp.tile([128, L * 4], F32)
            nc.sync.dma_start(out=g1c[:], in_=g1c_in[:])
            be1c = cp.tile([128, L * 4], F32)
            nc.sync.dma_start(out=be1c[:], in_=be1c_in[:])
            b2c = cp.tile([128, L], F32)
            nc.sync.dma_start(out=b2c[:], in_=b2c_in[:])
            lnwr = cp.tile([1, L * 128], F32)
            nc.sync.dma_start(out=lnwr[:], in_=lnwr_in[:])
            lnbr = cp.tile([1, L * 128], F32)
            nc.sync.dma_start(out=lnbr[:], in_=lnbr_in[:])
            epsc = cp.tile([128, 1], F32)
            nc.gpsimd.memset(epsc[:], EPS)

            x_fp = []
            for i in range(GPG):
                xfc = xp.tile([C, GPAD], F32, tag=f"xf{i}", name=f"xf{i}_0")
                nc.sync.dma_start(out=xfc[:], in_=xcn_in[:, i * GPAD:(i + 1) * GPAD])
                x_fp.append(xfc)

            tab0_b = dr.tile([NPAD, C], B16, tag="tab0b")
            nc.sync.dma_start(out=tab0_b[:], in_=xncbf_in[:])
            xtab_start = dr.tile([PTOT, C], B16, tag="xtab0", addr_space="Shared")
            if use_cc:
                nc.gpsimd.collective_compute(
                    "AllGather", OP.bypass, replica_groups=[list(range(NCORES))],
                    ins=[tab0_b[:].opt()], outs=[xtab_start[:].opt()])
            else:
                nc.sync.dma_start(out=xtab_start[0:NPAD, :], in_=tab0_b[:])
            agouts = []
            agins = []
            for l in range(nlayers):
                # ---------------- edge phase ----------------
                tab = xtab_start[:] if l == 0 else agouts[l - 1][:]
                dtab = xncbf_in if l == 0 else agins[l - 1][:]
                x2_fp = [xp.tile([C, GPAD], F32, tag=f"x2f{i}", bufs=2, name=f"x2f{i}_{l}") for i in range(GPG)]
                x2_bf = [xp.tile([C, GPAD], B16, tag=f"x2b{i}", bufs=2, name=f"x2b{i}_{l}") for i in range(GPG)]
                with (
                    tc.tile_pool(name="egath", bufs=2) as gp,
                    tc.tile_pool(name="eact", bufs=2) as ep,
                    tc.tile_pool(name="epsum", bufs=2, space="PSUM") as pp,
                    tc.tile_pool(name="aggpsum", bufs=2, space="PSUM") as ap_,
                ):
                    toff = 0
                    for b in range(NB):
                        tb = T_b[b]
                        ci, co = 128 * b // GPAD, 128 * b % GPAD
                        if tb == 0:
                            nc.vector.tensor_copy(x2_fp[ci][:, co:co + 128], x_fp[ci][:, co:co + 128])
                            nc.vector.tensor_copy(x2_bf[ci][:, co:co + 128], x_fp[ci][:, co:co + 128])
                            continue
                        ni = tb * 128
                        zdst = gp.tile([128, 1, ni], B16, tag="zdst")
                        nc.gpsimd.dma_gather(zdst[:], dtab, dstidx[:, toff * 8:(toff + tb) * 8],
                                             num_idxs=ni, num_idxs_reg=ni, elem_size=C, transpose=True, single_packet=False)
                        zsrc = gp.tile([128, 1, ni], B16, tag="zsrc")
                        nc.gpsimd.dma_gather(zsrc[:], tab, srcidx[:, toff * 8:(toff + tb) * 8],
                                             num_idxs=ni, num_idxs_reg=ni, elem_size=C, transpose=True, single_packet=False)
                        attr = gp.tile([65, ni], B16, tag="attr")
                        nc.sync.dma_start(out=attr[:], in_=attrt_in[:, toff * 128:toff * 128 + ni])
                        agg = ap_.tile([C, 128], F32, tag="agg", space="PSUM")
                        t0 = 0
                        while t0 < tb:
                            sgn = min(SG, tb - t0)
                            pre = pp.tile([128, SG, 2 * C], F32, tag="pre", space="PSUM")
                            for t in range(t0, t0 + sgn):
                                s = t - t0
                                nc.tensor.matmul(out=pre[:, s, :], lhsT=zdst[:, 0, t * 128:(t + 1) * 128],
                                                 rhs=wz[:, 3 * l + 0, :], start=True, stop=False)
                                nc.tensor.matmul(out=pre[:, s, :], lhsT=zsrc[:, 0, t * 128:(t + 1) * 128],
                                                 rhs=wz[:, 3 * l + 1, :], start=False, stop=False)
                                nc.tensor.matmul(out=pre[:, s, :], lhsT=attr[0:65, t * 128:(t + 1) * 128],
                                                 rhs=wz[0:65, 3 * l + 2, :], start=False, stop=True)
                            uv = ep.tile([128, SG, 256], F32, tag="uv")
                            sp = ep.tile([128, SG, 128], F32, tag="sp")
                            r = ep.tile([128, SG, 128], F32, tag="r")
                            msg = ep.tile([128, SG, 128], B16, tag="msg")
                            oh = ep.tile([128, SG, 128], B16, tag="oh")
                            nc.scalar.activation(uv[:, :sgn, :], pre[:, :sgn, :], AF.Exp)
                            nc.scalar.activation(sp[:, :sgn, :], uv[:, :sgn, C:2 * C], AF.Ln, bias=1.0)
                            nc.gpsimd.tensor_scalar_add(uv[:, :sgn, 0:C], uv[:, :sgn, 0:C], 1.0)
                            nc.vector.reciprocal_approx_fast(out=r[:, :sgn, :], in_=uv[:, :sgn, 0:C])
                            nc.vector.tensor_tensor(out=msg[:, :sgn, :], in0=sp[:, :sgn, :], in1=r[:, :sgn, :], op=OP.mult)
                            for t in range(t0, t0 + sgn):
                                s_ = t - t0
                                nc.vector.tensor_scalar(
                                    out=oh[:, s_, :], in0=iota[:, 0:128],
                                    scalar1=dstrel[:, toff + t:toff + t + 1], scalar2=None,
                                    op0=OP.is_equal)
                            for t in range(t0, t0 + sgn):
                                s = t - t0
                                nc.tensor.matmul(out=agg[:], lhsT=msg[:, s, :], rhs=oh[:, s, :],
                                                 start=(t == 0), stop=(t == tb - 1))
                            t0 += sgn
                        nc.vector.tensor_tensor(out=x2_fp[ci][:, co:co + 128],
                                                in0=x_fp[ci][:, co:co + 128], in1=agg[:], op=OP.add)
                        nc.vector.tensor_copy(x2_bf[ci][:, co:co + 128], x2_fp[ci][:, co:co + 128])
                        toff += tb

                if edge_only:
                    with tc.tile_pool(name="dbg", bufs=2, space="PSUM") as dbp:
                        for b in range(NB):
                            tp = dbp.tile([128, 128], F32, tag="dtp", space="PSUM")
                            nc.tensor.transpose(out=tp[:], in_=x2_fp[:, 128 * b:128 * (b + 1)], identity=ident[:])
                            xo = xp.tile([128, 128], F32, tag="dxo", bufs=2)
                            nc.scalar.activation(xo[:], tp[:], AF.Copy)
                            nc.sync.dma_start(out=xout[128 * b:128 * (b + 1), :], in_=xo[:])
                    break
                # ---------------- node phase (graph-major) ----------------
                with (
                    tc.tile_pool(name="nsb", bufs=1) as np_,
                    tc.tile_pool(name="nwork", bufs=3) as nw,
                    tc.tile_pool(name="npsum", bufs=1, space="PSUM") as npp,
                ):
                    h_bf = np_.tile([128, 4, NPAD], B16)
                    s1p = np_.tile([128, 4, GPG], F32)
                    s2p = np_.tile([128, 4, GPG], F32)
                    for g in range(GPG):
                        glo = g * GPAD
                        for k in range(4):
                            hp = npp.tile([128, GPAD], F32, tag="hpxp", space="PSUM", bufs=2)
                            for mlo in range(0, GPAD, NCH):
                                w = min(NCH, GPAD - mlo)
                                nc.tensor.matmul(out=hp[:, mlo:mlo + w], lhsT=w1t[:, 4 * l + k, :],
                                                 rhs=x2_bf[g][:, mlo:mlo + w], start=True, stop=True)
                            nc.vector.tensor_scalar(out=hp[:], in0=hp[:],
                                                    scalar1=b1c[:, 4 * l + k:4 * l + k + 1], scalar2=None, op0=OP.add)
                            nc.vector.tensor_tensor(out=h_bf[:, k, glo:glo + GPAD], in0=hp[:],
                                                    in1=mask[:, glo:glo + GPAD], op=OP.mult)
                            nc.vector.tensor_reduce(out=s1p[:, k, g:g + 1], in_=h_bf[:, k, glo:glo + GPAD],
                                                    axis=mybir.AxisListType.X, op=OP.add)
                            sq = nw.tile([128, GPAD], F32, tag="sq")
                            nc.scalar.activation(sq[:], h_bf[:, k, glo:glo + GPAD], AF.Square,
                                                 accum_out=s2p[:, k, g:g + 1])
                    bnstat = np_.tile([128, 8], F32)
                    nc.vector.tensor_reduce(out=bnstat[:, 0:4], in_=s1p[:], axis=mybir.AxisListType.X, op=OP.add)
                    nc.vector.tensor_reduce(out=bnstat[:, 4:8], in_=s2p[:], axis=mybir.AxisListType.X, op=OP.add)
                    bnin = dr.tile([128, 8], F32, tag="bnin", bufs=2)
                    bnout = dr.tile([128, 8], F32, tag="bnout", bufs=2, addr_space="Shared")
                    nc.sync.dma_start(out=bnin[:], in_=bnstat[:])
                    bns = np_.tile([128, 8], F32)
                    if use_cc:
                        nc.gpsimd.collective_compute(
                            "AllReduce", OP.add, replica_groups=[list(range(NCORES))],
                            ins=[bnin[:].opt()], outs=[bnout[:].opt()])
                        nc.sync.dma_start(out=bns[:], in_=bnout[:])
                    else:
                        nc.vector.tensor_scalar(out=bns[:], in0=bnstat[:], scalar1=float(NCORES), scalar2=None, op0=OP.mult)
                    mean = np_.tile([128, 4], F32)
                    nc.vector.tensor_scalar(out=mean[:], in0=bns[:, 0:4], scalar1=1.0 / meta["NN"], scalar2=None, op0=OP.mult)
                    var = np_.tile([128, 4], F32)
                    nc.vector.tensor_scalar(out=var[:], in0=bns[:, 4:8], scalar1=1.0 / meta["NN"], scalar2=None, op0=OP.mult)
                    msq = np_.tile([128, 4], F32)
                    nc.vector.tensor_tensor(out=msq[:], in0=mean[:], in1=mean[:], op=OP.mult)
                    nc.vector.tensor_tensor(out=var[:], in0=var[:], in1=msq[:], op=OP.subtract)
                    rstd = np_.tile([128, 4], F32)
                    nc.scalar.activation(rstd[:], var[:], AF.Ln, bias=epsc[:])
                    nc.scalar.activation(rstd[:], rstd[:], AF.Exp, scale=-0.5)
                    a_bn = np_.tile([128, 4], F32)
                    nc.vector.tensor_tensor(out=a_bn[:], in0=rstd[:], in1=g1c[:, 4 * l:4 * l + 4], op=OP.mult)
                    b_bn = np_.tile([128, 4], F32)
                    nc.vector.tensor_tensor(out=b_bn[:], in0=mean[:], in1=a_bn[:], op=OP.mult)
                    nc.vector.tensor_tensor(out=b_bn[:], in0=be1c[:, 4 * l:4 * l + 4], in1=b_bn[:], op=OP.subtract)

                    if l < nlayers - 1:
                        agin = dr.tile([NPAD, C], B16, tag="agin", bufs=2)
                        agout = dr.tile([PTOT, C], B16, tag="agout", bufs=2, addr_space="Shared")
                        agins.append(agin)
                    y_fp = [xp.tile([C, GPAD], F32, tag=f"xf{i}", bufs=2, name=f"yf{i}_{l}") for i in range(GPG)]
                    for g in range(GPG):
                        glo = g * GPAD
                        xpp = npp.tile([128, GPAD], F32, tag="hpxp", space="PSUM", bufs=2)
                        for k in range(4):
                            hn = nw.tile([128, GPAD], B16, tag="hn")
                            nc.scalar.activation(hn[:], h_bf[:, k, glo:glo + GPAD], AF.Relu,
                                                 scale=a_bn[:, k:k + 1], bias=b_bn[:, k:k + 1])
                            for mlo in range(0, GPAD, NCH):
                                w = min(NCH, GPAD - mlo)
                                nc.tensor.matmul(out=xpp[:, mlo:mlo + w], lhsT=w2t[:, 4 * l + k, :],
                                                 rhs=hn[:, mlo:mlo + w], start=(k == 0), stop=(k == 3))
                        t1 = nw.tile([128, GPAD], F32, tag="t1n")
                        nc.vector.tensor_scalar(out=t1[:], in0=xpp[:],
                                                scalar1=b2c[:, l:l + 1], scalar2=None, op0=OP.add)
                        nc.vector.tensor_tensor(out=t1[:], in0=t1[:], in1=x2_fp[g][:], op=OP.add)
                        x3g = nw.tile([128, GPAD], F32, tag="x3g", bufs=2)
                        nc.vector.tensor_tensor(out=x3g[:], in0=t1[:], in1=mask[:, glo:glo + GPAD], op=OP.mult)
                        # LN stats for this graph only
                        lnp = np_.tile([128, 2], F32, tag="lnp", bufs=2)
                        nc.vector.tensor_reduce(out=lnp[:, 0:1], in_=x3g[:], axis=mybir.AxisListType.X, op=OP.add)
                        sqg = nw.tile([128, GPAD], F32, tag="sqg")
                        nc.scalar.activation(sqg[:], x3g[:], AF.Square, accum_out=lnp[:, 1:2])
                        lnt = np_.tile([1, 2], F32, tag="lnt", bufs=2)
                        nc.gpsimd.tensor_reduce(out=lnt[:], in_=lnp[:], axis=mybir.AxisListType.C, op=OP.add)
                        mv = np_.tile([1, 2], F32, tag="mv", bufs=2)
                        nc.vector.tensor_scalar(out=mv[:], in0=lnt[:], scalar1=invcnt[:, g:g + 1], scalar2=None, op0=OP.mult)
                        m2g = np_.tile([1, 1], F32, tag="m2g", bufs=2)
                        nc.vector.tensor_tensor(out=m2g[:], in0=mv[:, 0:1], in1=mv[:, 0:1], op=OP.mult)
                        vgg = np_.tile([1, 1], F32, tag="vgg", bufs=2)
                        nc.vector.tensor_tensor(out=vgg[:], in0=mv[:, 1:2], in1=m2g[:], op=OP.subtract)
                        rgg = np_.tile([1, 1], F32, tag="rgg", bufs=2)
                        nc.scalar.activation(rgg[:], vgg[:], AF.Ln, bias=epsc[0:1, :])
                        nc.scalar.activation(rgg[:], rgg[:], AF.Exp, scale=-0.5)
                        bgg = np_.tile([1, 1], F32, tag="bgg", bufs=2)
                        nc.vector.tensor_tensor(out=bgg[:], in0=mv[:, 0:1], in1=rgg[:], op=OP.mult)
                        nc.vector.tensor_scalar(out=bgg[:], in0=bgg[:], scalar1=-1.0, scalar2=None, op0=OP.mult)
                        arow = np_.tile([1, GPAD], F32, tag="arow", bufs=2)
                        brow = np_.tile([1, GPAD], F32, tag="brow", bufs=2)
                        nc.vector.tensor_scalar(out=arow[:], in0=ones[:, glo:glo + GPAD],
                                                scalar1=rgg[:], scalar2=None, op0=OP.mult)
                        nc.vector.tensor_scalar(out=brow[:], in0=ones[:, glo:glo + GPAD],
                                                scalar1=bgg[:], scalar2=None, op0=OP.mult)
                        for mlo in range(0, GPAD, NCH):
                            w = min(NCH, GPAD - mlo)
                            A = npp.tile([128, NCH], F32, tag="A", space="PSUM")
                            B = npp.tile([128, NCH], F32, tag="B", space="PSUM")
                            nc.tensor.matmul(out=A[:, :w], lhsT=lnwr[:, 128 * l:128 * (l + 1)],
                                             rhs=arow[:, mlo:mlo + w], start=True, stop=True)
                            nc.tensor.matmul(out=B[:, :w], lhsT=lnwr[:, 128 * l:128 * (l + 1)],
                                             rhs=brow[:, mlo:mlo + w], start=True, stop=False)
                            nc.tensor.matmul(out=B[:, :w], lhsT=lnbr[:, 128 * l:128 * (l + 1)],
                                             rhs=ones[:, glo + mlo:glo + mlo + w], start=False, stop=True)
                            t2 = nw.tile([128, NCH], F32, tag="t2n")
                            nc.vector.tensor_tensor(out=t2[:, :w], in0=x3g[:, mlo:mlo + w], in1=A[:, :w], op=OP.mult)
                            nc.vector.tensor_tensor(out=y_fp[g][:, mlo:mlo + w], in0=t2[:, :w], in1=B[:, :w], op=OP.add)
                        # transpose this graph's blocks and ship them
                        for bb in range(GPAD // 128):
                            gb = glo + 128 * bb
                            tp = npp.tile([128, 128], F32, tag="tp", space="PSUM", bufs=2)
                            nc.tensor.transpose(out=tp[:], in_=y_fp[g][:, 128 * bb:128 * bb + 128], identity=ident[:])
                            if l < nlayers - 1:
                                xnc = nw.tile([128, 128], B16, tag="xnc")
                                nc.scalar.activation(xnc[:], tp[:], AF.Copy)
                                nc.sync.dma_start(out=agin[gb:gb + 128, :], in_=xnc[:])
                            else:
                                xnc32 = nw.tile([128, 128], F32, tag="xnc32")
                                nc.scalar.activation(xnc32[:], tp[:], AF.Copy)
                                nc.sync.dma_start(out=xout[gb:gb + 128, :], in_=xnc32[:])
                    if l < nlayers - 1:
                        if use_cc:
                            nc.gpsimd.collective_compute(
                                "AllGather", OP.bypass, replica_groups=[list(range(NCORES))],
                                ins=[agin[:].opt()], outs=[agout[:].opt()])
                            agouts.append(agout)
                        else:
                            agouts.append(agout)
                            nc.sync.dma_start(out=agout[0:NPAD, :], in_=agin[:])
                x_fp = y_fp

    nc.finalize()
    return nc


_CACHE = {}


def kernel(x, node_batch, edge_index, edge_attr,
           Wf, bf, Ws, bs, W1, b1, g1, be1, W2, b2, lnw, lnb):
    from concourse.bass_utils import run_bass_kernel_spmd

    per_core, meta = _preprocess(x, node_batch, edge_index, edge_attr)
    wd = _prep_weights(Wf, bf, Ws, bs, W1, b1, g1, be1, W2, b2, lnw, lnb)
    key = (meta["NPAD"], meta["NN"], tuple(meta["T_b"]))
    if key not in _CACHE:
        _CACHE[key] = _trace(meta)
    nc = _CACHE[key]

    iota = np.ascontiguousarray(
        np.broadcast_to(np.arange(128, dtype=np.float32), (SG, 128, 128)).transpose(1, 0, 2)
        .reshape(128, SG * 128)).astype(BF16)
    ident = np.eye(128, dtype=np.float32)
    ones = np.ones((1, meta["NPAD"]), np.float32)
    in_maps = []
    for c in range(NCORES):
        m = dict(per_core[c])
        m.update(wd)
        m.update(iota=iota, ident=ident, ones=ones)
        in_maps.append(m)
    res = run_bass_kernel_spmd(nc, in_maps, list(range(NCORES)))

    pad_slot = meta["pad_slot"]
    NPAD = meta["NPAD"]
    out = np.zeros((meta["NN"], C), np.float32)
    for c in range(NCORES):
        own = (pad_slot >= c * NPAD) & (pad_slot < (c + 1) * NPAD)
        out[own] = res.results[c]["xout"][pad_slot[own] - c * NPAD]
    return out

